# revision 31
# baseline (speedup 1.0000x reference)
"""CapsuleLayer kernel for 8 trn2 NeuronCores.

Math (from the reference):
    c        = softmax(bias[0,:,:,0,0], axis=1)            # [I, J]
    s[b,j,d] = sum_{i,p} x[b,i,p] * W[i,j,p,d] * c[i,j]    # [B, J, D]
    out      = squash(s, axis=-1)

Folding c into W gives one big matmul
    s = X @ Wc,  X: [B, K], Wc: [K, N],  K = I*P = 32768, N = J*D = 1024.

Sharding: split the contraction dim K across the 8 cores (each core reads a
distinct 1/8 slice of W, so W is read exactly once fleet-wide — the memory
roofline optimum). Each core computes a partial [*, N] sum; the host adds
the 8 partials and applies the tiny squash.

Precision/speed (MODE) — this problem family gates at rel_err < 2e-2:
  "e3m4"  — W cast to fp8 e3m4 (TRN FP8_EXP3, 4 mantissa bits), x split
            into e3m4 hi+lo packed side by side into one 128-wide
            stationary operand, so ONE pass of W through the PE computes
            both the hi and lo partial sums (out partitions 0-63 = hi,
            64-127 = lo; host adds hi + lo/SL). W-quantization error
            measured 1.34e-2 on HW (x error is negligible thanks to the
            split; e3m4 subnormals verified exact on the PE).
            Halves the DMA bytes vs fp16: ~4.7 MB/core (~13.5 us at the
            measured 340-390 GB/s stream rate) balanced against the PE
            floor of 32768 moving columns = 13.7 us — a true ridge.
  "fp16x1"— x and Wc cast to fp16 (Wc pre-scaled by 2^8). Error 3.0e-4,
            9 MB/core DMA, ~25 us/core roofline.

Layout: one input tensor per core, K-tile-major: each 128-row K-tile packs
[xh | xl | W] as contiguous columns, so a chunked DMA stream feeds
everything. Chunk issues alternate between the Sync and Scalar HWDGE rings
(descriptors queue 2x faster during the ramp) and are all emitted before
any eviction dma_start (engines execute their queues in order — a waiting
eviction issue must not block later chunk issues; that mistake cost 2.7us).

Schedule (measured on HW, times relative to kernel start at ~2.4 GHz):
  ~6.5us  fixed Bass prologue (runtime start, engine preambles, barriers)
  ~6.9us  first chunk DMA issued; data flows 8.4-22us at ~340-390 GB/s
  ~7.0us  30 small dummy matmuls on a memset tile keep the PE busy until
          chunk 0 is consumable AND fill the ~3.4us HAM activity window,
          so real matmuls run at 2.4 GHz (216 ns / 512-col MM) from the
          very first one (verified: zero cold real matmuls in the trace)
  ~11us   64 real matmuls, PE-bound, back-to-back
  ~21us   psA (K-tiles 0-15) eviction rides mid-stream, fully hidden
  ~T-0.9  psB bank 0 evicted during bank 1's matmul tail (bank-major last
          chunk); only psB bank 1's copy+store chain (~2.3us) is exposed
  +~2.6us fixed Bass epilogue (drain, sem clear, barriers)

Known run-to-run noise (not controllable from the kernel): engine start
jitter +-1.5us, PE clock 2.2 vs 2.4 GHz power states, DMA stream rate
290-390 GB/s under 8-core HBM contention.
"""

import ml_dtypes
import numpy as np

import concourse.mybir as mybir
import concourse.tile as tile
from concourse import bacc
from concourse.bass_utils import run_bass_kernel_spmd

MODE = "e3m4"          # "e3m4" | "fp16x1"

# Problem shapes (hardcoded per contract).
B, I, P, J, D = 64, 2048, 16, 32, 32
K = I * P            # 32768 contraction
N = J * D            # 1024 output features
N_CORES = 8
K_CORE = K // N_CORES  # 4096 contraction rows per core
KT = 128               # K-tile (partition dim of one matmul)
NKT = K_CORE // KT     # 32 K-tiles per core
# Tapered DMA chunk sizes (in K-tiles), summing to NKT. Small head chunks
# start the PE early (the kernel is PE-bound, so the real-MM stream's start
# is the critical path); big tail chunks keep the per-partition DMA runs
# long (better descriptor efficiency) once the DMA is ahead of the PE.
# The first 5 chunks cover K-tiles 0..15 exactly = the psA half (see below).
# Back half uses 3-tile chunks: when a slow-DMA run starves the PE, each
# stall's resume point is a chunk-completion semaphore — finer chunks mean
# the PE waits for less data (and less receipt lag) per stall.
CHUNKS = [2, 2, 4, 4, 4, 4, 3, 3, 3, 3]
KSPLIT = 16            # K-tiles 0..15 accumulate in psA, 16..31 in psB; psA
                       # is evicted mid-stream while the PE works on psB, so
                       # only psB's eviction sits in the tail.
NB = N // 512          # PSUM-bank-sized slices of N (bank = 512 fp32)
N_WARM = 30            # 128-col dummy matmuls (~107ns each cold) that keep
                       # the PE busy from the earliest post-barrier moment
                       # until chunk 0's data is consumable (~3.4us) — this
                       # both fills the HAM activity window (so real
                       # matmuls run at 2.4 GHz from the start) and costs
                       # nothing, because the PE has no real data to chew
                       # on during that window.

if MODE == "e3m4":
    LW = 128           # stationary width: [xh 64 | xl 64]
    NP_DTYPE = ml_dtypes.float8_e3m4
    MM_DTYPE = mybir.dt.float8e3
    OUT_DTYPE = mybir.dt.float16
    OUT_NP = np.float16
    W_SCALE = 1024.0   # power-of-2 lift; clip to e3m4 max normal 15.5
    XL_SCALE = 64.0    # power-of-2 lift for the x residual term
else:
    LW = B             # stationary = x only
    NP_DTYPE = np.float16
    MM_DTYPE = mybir.dt.float16
    OUT_DTYPE = mybir.dt.float32
    OUT_NP = np.float32
    W_SCALE = 256.0    # exact power-of-2 lift keeping fp16(Wc) normal
    XL_SCALE = None

TC = LW + N            # packed columns per K-tile

_NC_CACHE = None


def _build_nc():
    """Per-core program: out[LW,N] accumulated over 32 K-tiles in PSUM."""
    nc = bacc.Bacc(trn_type="TRN2", target_bir_lowering=False, debug=False)
    f32 = mybir.dt.float32

    wx = nc.dram_tensor("wx", [KT, NKT * TC], MM_DTYPE, kind="ExternalInput")
    out = nc.dram_tensor("out", [LW, 2 * N], OUT_DTYPE, kind="ExternalOutput")

    assert sum(CHUNKS) == NKT
    n_small = sum(1 for s in CHUNKS if s <= 2)
    n_big = sum(1 for s in CHUNKS if s > 2)
    with tile.TileContext(nc) as tc:
        with (
            tc.tile_pool(name="cpool", bufs=1) as cpool,
            # One buffer per chunk (no slot reuse) so every chunk DMA can be
            # in flight at once; small/big pools so slots aren't all padded
            # to the largest chunk (SBUF budget).
            tc.tile_pool(name="wsmall", bufs=max(n_small, 1)) as wsmall,
            tc.tile_pool(name="wbig", bufs=max(n_big, 1)) as wbig,
            tc.tile_pool(name="opool", bufs=1) as opool,
            tc.tile_pool(name="pspool", bufs=1, space="PSUM") as pspool,
        ):
            # HAM warm-up: PE must stay busy ~3.4us to reach 2.4 GHz. These
            # dummies depend only on a memset tile, so they run during the
            # first chunk's DMA flight.
            # Small warm tile: the memset is ~130ns (vs ~480 for 512 cols),
            # so the PE's HAM-window fill starts as early as Vector can
            # release it; 128-col dummies give fine-grained granularity so
            # the dummy stream ends close to chunk 0 becoming consumable.
            warm = cpool.tile([KT, 128], MM_DTYPE)
            nc.vector.memset(warm[:], 1.0)
            warm_ps = pspool.tile([KT, 128], f32)
            for _ in range(N_WARM):
                nc.tensor.matmul(
                    warm_ps[:], warm[:], warm[:], start=True, stop=True
                )

            # One PSUM tile per (K-half, bank) so Tile's dependency tracking
            # releases each eviction at that accumulator's own last matmul.
            psA = [pspool.tile([LW, 512], f32, name=f"psA{b}") for b in range(NB)]
            psB = [pspool.tile([LW, 512], f32, name=f"psB{b}") for b in range(NB)]
            o_sb = opool.tile([LW, 2 * N], OUT_DTYPE)
            o_scr = opool.tile([1, 2], OUT_DTYPE)

            def emit_evict(ps_pair, ocol, tail):
                # Copies: Scalar takes bank 0, Vector bank 1 (parallel).
                # Stores: mid-stream (psA) both go on the idle Sync ring so
                # Scalar's queue stays clear for the tail; in the tail (psB)
                # Scalar stores its own bank 0 (same-engine, no wake
                # latency) while Sync stores bank 1.
                b1 = ocol + 512
                nc.scalar.copy(o_sb[:, ocol : ocol + 512], ps_pair[0][:])
                if tail:
                    nc.scalar.dma_start(
                        out.ap()[:, ocol : ocol + 512], o_sb[:, ocol : ocol + 512]
                    )
                else:
                    nc.sync.dma_start(
                        out.ap()[:, ocol : ocol + 512], o_sb[:, ocol : ocol + 512]
                    )
                nc.vector.tensor_copy(o_sb[:, b1 : b1 + 512], ps_pair[1][:])
                nc.sync.dma_start(
                    out.ap()[:, b1 : b1 + 512], o_sb[:, b1 : b1 + 512]
                )

            t = 0
            col = 0
            for ci, csz in enumerate(CHUNKS):
                pool = wsmall if csz <= 2 else wbig
                w_sb = pool.tile([KT, csz * TC], MM_DTYPE)
                # Alternate the two HWDGE rings (Sync and Scalar) so chunk
                # descriptors enter the SDMA queues twice as fast during the
                # ramp. All chunk issues are emitted before any eviction
                # dma_start so neither engine's in-order queue can stall the
                # input stream on an eviction's semaphore wait.
                eng = nc.sync if ci % 2 == 0 else nc.scalar
                eng.dma_start(w_sb[:], wx.ap()[:, col : col + csz * TC])
                col += csz * TC
                if ci < len(CHUNKS) - 1:
                    # lhsT-major groups pair weight loads.
                    for tl in range(csz):
                        kt = t + tl
                        ps = psA if kt < KSPLIT else psB
                        base = tl * TC
                        lhsT = w_sb[:, base : base + LW]
                        for nb in range(NB):
                            nc.tensor.matmul(
                                ps[nb][:],
                                lhsT,
                                w_sb[:, base + LW + nb * 512 : base + LW + (nb + 1) * 512],
                                start=(kt in (0, KSPLIT)),
                                stop=(kt == KSPLIT - 1),
                            )
                else:
                    # Last chunk goes bank-major so psB bank 0 finishes a
                    # whole chunk early and its Scalar eviction chain hides
                    # under psB bank 1's matmul tail.
                    for nb in range(NB):
                        for tl in range(csz):
                            base = tl * TC
                            nc.tensor.matmul(
                                psB[nb][:],
                                w_sb[:, base : base + LW],
                                w_sb[:, base + LW + nb * 512 : base + LW + (nb + 1) * 512],
                                start=False,
                                stop=(tl == csz - 1),
                            )
                t += csz

            # psA's eviction is hidden behind psB's matmul work; only psB's
            # eviction sits in the tail. Both are emitted after every chunk
            # issue (see note above). The 1-element dependency copies from
            # the LAST chunk's buffer hold psA's stores until the whole
            # input stream has landed: its HBM writes would otherwise
            # interleave with the input stream's tail exactly when the
            # PE-supply margin is thinnest (431 vs 432 ns per K-tile).
            nc.scalar.copy(o_scr[:, 0:1], w_sb[0:1, 0:1])
            nc.vector.tensor_copy(o_scr[:, 1:2], w_sb[0:1, 1:2])
            emit_evict(psA, 0, tail=False)
            emit_evict(psB, N, tail=True)
    # Run Bacc's compile pipeline (wait legalization, register allocation).
    # run_bass_via_pjrt serializes nc.m as-is and never finalizes.
    nc.finalize()
    return nc


def _get_nc():
    global _NC_CACHE
    if _NC_CACHE is None:
        _NC_CACHE = _build_nc()
    return _NC_CACHE


def _prepare_in_maps(inputs: np.ndarray, W: np.ndarray, bias: np.ndarray):
    """Fold softmax(bias) into W, split precision, pack K-tile-major."""
    x = np.asarray(inputs, dtype=np.float32)
    Wf = np.asarray(W, dtype=np.float32)
    b = np.asarray(bias, dtype=np.float32)[0, :, :, 0, 0]          # [I, J]

    # softmax over J per input capsule i (fp32, matches jax.nn.softmax).
    m = b.max(axis=1, keepdims=True)
    e = np.exp(b - m)
    c = e / e.sum(axis=1, keepdims=True)                            # [I, J]

    # Wc[(i,p),(j,d)] = W[i,j,p,d] * c[i,j]  ->  [K, N]
    wc = (Wf.transpose(0, 2, 1, 3) * c[:, None, :, None]).reshape(K, N)
    xT = np.ascontiguousarray(x.reshape(B, K).T)                    # [K, B]

    packed = np.empty((K, TC), dtype=NP_DTYPE)
    if MODE == "e3m4":
        xh = xT.astype(NP_DTYPE)
        resid = (xT - xh.astype(np.float32)) * np.float32(XL_SCALE)
        packed[:, 0:B] = xh
        packed[:, B : 2 * B] = resid.astype(NP_DTYPE)
        ws = np.clip(wc * np.float32(W_SCALE), -15.5, 15.5)
        packed[:, LW:] = ws.astype(NP_DTYPE)
    else:
        packed[:, 0:B] = xT.astype(NP_DTYPE)
        packed[:, LW:] = (wc * np.float32(W_SCALE)).astype(NP_DTYPE)

    in_maps = []
    for cid in range(N_CORES):
        sl = slice(cid * K_CORE, (cid + 1) * K_CORE)
        # K-tile-major packing: [NKT, KT, TC] -> [KT, NKT*TC]
        core = np.ascontiguousarray(
            packed[sl].reshape(NKT, KT, TC).swapaxes(0, 1).reshape(KT, NKT * TC)
        )
        in_maps.append({"wx": core})
    return in_maps


def _squash(s: np.ndarray) -> np.ndarray:
    s2 = np.sum(np.square(s), axis=-1, keepdims=True, dtype=np.float32)
    scale = s2 / (1.0 + s2) / np.sqrt(s2)
    return (scale * s).astype(np.float32)


def run(inputs, W, bias, **spmd_kwargs):
    """Full pipeline; returns (output, BassKernelResults)."""
    in_maps = _prepare_in_maps(inputs, W, bias)
    try:
        res = run_bass_kernel_spmd(
            _get_nc(), in_maps, core_ids=list(range(N_CORES)), **spmd_kwargs
        )
    except Exception:
        # A crashed prior process can leave a core wedged
        # (NRT_EXEC_UNIT_UNRECOVERABLE); one retry clears it.
        import time
        time.sleep(2.0)
        res = run_bass_kernel_spmd(
            _get_nc(), in_maps, core_ids=list(range(N_CORES)), **spmd_kwargs
        )
    s = np.zeros((LW, N), dtype=np.float32)
    for r in res.results:
        o = np.asarray(r["out"], dtype=np.float32)
        s += o[:, 0:N] + o[:, N : 2 * N]
    if MODE == "e3m4":
        s = s[0:B] + s[B : 2 * B] / np.float32(XL_SCALE)
    s /= np.float32(W_SCALE)
    out = _squash(s.reshape(B, J, D))
    return out, res


def kernel(inputs, W, bias):
    out, _ = run(inputs, W, bias)
    return out


# revision 32
# speedup vs baseline: 1.0571x; 1.0571x over previous
"""CapsuleLayer kernel for 8 trn2 NeuronCores.

Math (from the reference):
    c        = softmax(bias[0,:,:,0,0], axis=1)            # [I, J]
    s[b,j,d] = sum_{i,p} x[b,i,p] * W[i,j,p,d] * c[i,j]    # [B, J, D]
    out      = squash(s, axis=-1)

Folding c into W gives one big matmul
    s = X @ Wc,  X: [B, K], Wc: [K, N],  K = I*P = 32768, N = J*D = 1024.

Sharding: split the contraction dim K across the 8 cores (each core reads a
distinct 1/8 slice of W, so W is read exactly once fleet-wide — the memory
roofline optimum). Each core computes a partial [*, N] sum; the host adds
the 8 partials and applies the tiny squash.

Precision/speed (MODE) — this problem family gates at rel_err < 2e-2:
  "e3m4"  — W cast to fp8 e3m4 (TRN FP8_EXP3, 4 mantissa bits), x split
            into e3m4 hi+lo packed side by side into one 128-wide
            stationary operand, so ONE pass of W through the PE computes
            both the hi and lo partial sums (out partitions 0-63 = hi,
            64-127 = lo; host adds hi + lo/SL). W-quantization error
            measured 1.34e-2 on HW (x error is negligible thanks to the
            split; e3m4 subnormals verified exact on the PE).
            Halves the DMA bytes vs fp16: ~4.7 MB/core (~13.5 us at the
            measured 340-390 GB/s stream rate) balanced against the PE
            floor of 32768 moving columns = 13.7 us — a true ridge.
  "fp16x1"— x and Wc cast to fp16 (Wc pre-scaled by 2^8). Error 3.0e-4,
            9 MB/core DMA, ~25 us/core roofline.

Layout: one input tensor per core, K-tile-major: each 128-row K-tile packs
[xh | xl | W] as contiguous columns, so a chunked DMA stream feeds
everything. Chunk issues alternate between the Sync and Scalar HWDGE rings
(descriptors queue 2x faster during the ramp) and are all emitted before
any eviction dma_start (engines execute their queues in order — a waiting
eviction issue must not block later chunk issues; that mistake cost 2.7us).

Schedule (measured on HW, times relative to kernel start at ~2.4 GHz):
  ~6.5us  fixed Bass prologue (runtime start, engine preambles, barriers)
  ~6.9us  first chunk DMA issued; data flows 8.4-22us at ~340-390 GB/s
  ~7.0us  30 small dummy matmuls on a memset tile keep the PE busy until
          chunk 0 is consumable AND fill the ~3.4us HAM activity window,
          so real matmuls run at 2.4 GHz (216 ns / 512-col MM) from the
          very first one (verified: zero cold real matmuls in the trace)
  ~11us   64 real matmuls, PE-bound, back-to-back
  ~21us   psA (K-tiles 0-15) eviction rides mid-stream, fully hidden
  ~T-0.9  psB bank 0 evicted during bank 1's matmul tail (bank-major last
          chunk); only psB bank 1's copy+store chain (~2.3us) is exposed
  +~2.6us fixed Bass epilogue (drain, sem clear, barriers)

Known run-to-run noise (not controllable from the kernel): engine start
jitter +-1.5us, PE clock 2.2 vs 2.4 GHz power states, DMA stream rate
290-390 GB/s under 8-core HBM contention.
"""

import ml_dtypes
import numpy as np

import concourse.mybir as mybir
import concourse.tile as tile
from concourse import bacc
from concourse.bass_utils import run_bass_kernel_spmd

MODE = "e3m4"          # "e3m4" | "fp16x1"

# Problem shapes (hardcoded per contract).
B, I, P, J, D = 64, 2048, 16, 32, 32
K = I * P            # 32768 contraction
N = J * D            # 1024 output features
N_CORES = 8
K_CORE = K // N_CORES  # 4096 contraction rows per core
KT = 128               # K-tile (partition dim of one matmul)
NKT = K_CORE // KT     # 32 K-tiles per core
# Tapered DMA chunk sizes (in K-tiles), summing to NKT. Small head chunks
# start the PE early (the kernel is PE-bound, so the real-MM stream's start
# is the critical path); big tail chunks keep the per-partition DMA runs
# long (better descriptor efficiency) once the DMA is ahead of the PE.
# The first 5 chunks cover K-tiles 0..15 exactly = the psA half (see below).
# Back half uses 3-tile chunks: when a slow-DMA run starves the PE, each
# stall's resume point is a chunk-completion semaphore — finer chunks mean
# the PE waits for less data (and less receipt lag) per stall.
CHUNKS = [2, 2, 4, 4, 4, 4, 3, 3, 3, 3]
KSPLIT = 16            # K-tiles 0..15 accumulate in psA, 16..31 in psB; psA
                       # is evicted mid-stream while the PE works on psB, so
                       # only psB's eviction sits in the tail.
NB = N // 512          # PSUM-bank-sized slices of N (bank = 512 fp32)
N_WARM = 30            # 128-col dummy matmuls (~107ns each cold) that keep
                       # the PE busy from the earliest post-barrier moment
                       # until chunk 0's data is consumable (~3.4us) — this
                       # both fills the HAM activity window (so real
                       # matmuls run at 2.4 GHz from the start) and costs
                       # nothing, because the PE has no real data to chew
                       # on during that window.

if MODE == "e3m4":
    LW = 128           # stationary width: [xh 64 | xl 64]
    NP_DTYPE = ml_dtypes.float8_e3m4
    MM_DTYPE = mybir.dt.float8e3
    OUT_DTYPE = mybir.dt.float16
    OUT_NP = np.float16
    W_SCALE = 1024.0   # power-of-2 lift; clip to e3m4 max normal 15.5
    XL_SCALE = 64.0    # power-of-2 lift for the x residual term
else:
    LW = B             # stationary = x only
    NP_DTYPE = np.float16
    MM_DTYPE = mybir.dt.float16
    OUT_DTYPE = mybir.dt.float32
    OUT_NP = np.float32
    W_SCALE = 256.0    # exact power-of-2 lift keeping fp16(Wc) normal
    XL_SCALE = None

TC = LW + N            # packed columns per K-tile

_NC_CACHE = None


def _build_nc():
    """Per-core program: out[LW,N] accumulated over 32 K-tiles in PSUM."""
    nc = bacc.Bacc(trn_type="TRN2", target_bir_lowering=False, debug=False)
    f32 = mybir.dt.float32

    wx = nc.dram_tensor("wx", [KT, NKT * TC], MM_DTYPE, kind="ExternalInput")
    out = nc.dram_tensor("out", [LW, 2 * N], OUT_DTYPE, kind="ExternalOutput")

    assert sum(CHUNKS) == NKT
    n_small = sum(1 for s in CHUNKS if s <= 2)
    n_big = sum(1 for s in CHUNKS if s > 2)
    with tile.TileContext(nc) as tc:
        with (
            tc.tile_pool(name="cpool", bufs=1) as cpool,
            # One buffer per chunk (no slot reuse) so every chunk DMA can be
            # in flight at once; small/big pools so slots aren't all padded
            # to the largest chunk (SBUF budget).
            tc.tile_pool(name="wsmall", bufs=max(n_small, 1)) as wsmall,
            tc.tile_pool(name="wbig", bufs=max(n_big, 1)) as wbig,
            tc.tile_pool(name="opool", bufs=1) as opool,
            tc.tile_pool(name="pspool", bufs=1, space="PSUM") as pspool,
        ):
            # HAM warm-up: PE must stay busy ~3.4us to reach 2.4 GHz. These
            # dummies depend only on a memset tile, so they run during the
            # first chunk's DMA flight.
            # Small warm tile: the memset is ~130ns (vs ~480 for 512 cols),
            # so the PE's HAM-window fill starts as early as Vector can
            # release it; 128-col dummies give fine-grained granularity so
            # the dummy stream ends close to chunk 0 becoming consumable.
            warm = cpool.tile([KT, 128], MM_DTYPE)
            nc.vector.memset(warm[:], 1.0)
            warm_ps = pspool.tile([KT, 128], f32)
            for _ in range(N_WARM):
                nc.tensor.matmul(
                    warm_ps[:], warm[:], warm[:], start=True, stop=True
                )

            # One PSUM tile per (K-half, bank) so Tile's dependency tracking
            # releases each eviction at that accumulator's own last matmul.
            psA = [pspool.tile([LW, 512], f32, name=f"psA{b}") for b in range(NB)]
            psB = [pspool.tile([LW, 512], f32, name=f"psB{b}") for b in range(NB)]
            o_sb = opool.tile([LW, 2 * N], OUT_DTYPE)
            o_scr = opool.tile([1, 2], OUT_DTYPE)

            def emit_evict(ps_pair, ocol, tail):
                # Copies: Scalar takes bank 0, Vector bank 1 (parallel).
                # Stores: mid-stream (psA) both go on the idle Sync ring so
                # Scalar's queue stays clear for the tail; in the tail (psB)
                # Scalar stores its own bank 0 (same-engine, no wake
                # latency) while Sync stores bank 1.
                b1 = ocol + 512
                nc.scalar.copy(o_sb[:, ocol : ocol + 512], ps_pair[0][:])
                if tail:
                    nc.scalar.dma_start(
                        out.ap()[:, ocol : ocol + 512], o_sb[:, ocol : ocol + 512]
                    )
                else:
                    nc.sync.dma_start(
                        out.ap()[:, ocol : ocol + 512], o_sb[:, ocol : ocol + 512]
                    )
                nc.vector.tensor_copy(o_sb[:, b1 : b1 + 512], ps_pair[1][:])
                nc.sync.dma_start(
                    out.ap()[:, b1 : b1 + 512], o_sb[:, b1 : b1 + 512]
                )

            t = 0
            col = 0
            for ci, csz in enumerate(CHUNKS):
                pool = wsmall if csz <= 2 else wbig
                w_sb = pool.tile([KT, csz * TC], MM_DTYPE)
                # All input chunks go on ONE HWDGE ring (Sync): the 16 SDMA
                # engines drain concurrent rings round-robin at packet
                # granularity, so chunks split across two rings transfer in
                # parallel at half rate each and the FIRST completion —
                # which gates the PE — lands a whole chunk later. Serial
                # single-ring order gives the earliest per-chunk completion
                # times, and issue throughput is not a constraint (10
                # issues x 0.65us finish well before the data needs them).
                # All chunk issues are emitted before any eviction dma_start
                # so Sync's in-order queue can never stall the input stream
                # on an eviction's semaphore wait.
                nc.sync.dma_start(w_sb[:], wx.ap()[:, col : col + csz * TC])
                col += csz * TC
                if ci < len(CHUNKS) - 1:
                    # lhsT-major groups pair weight loads.
                    for tl in range(csz):
                        kt = t + tl
                        ps = psA if kt < KSPLIT else psB
                        base = tl * TC
                        lhsT = w_sb[:, base : base + LW]
                        for nb in range(NB):
                            nc.tensor.matmul(
                                ps[nb][:],
                                lhsT,
                                w_sb[:, base + LW + nb * 512 : base + LW + (nb + 1) * 512],
                                start=(kt in (0, KSPLIT)),
                                stop=(kt == KSPLIT - 1),
                            )
                else:
                    # Last chunk goes bank-major so psB bank 0 finishes a
                    # whole chunk early and its Scalar eviction chain hides
                    # under psB bank 1's matmul tail.
                    for nb in range(NB):
                        for tl in range(csz):
                            base = tl * TC
                            nc.tensor.matmul(
                                psB[nb][:],
                                w_sb[:, base : base + LW],
                                w_sb[:, base + LW + nb * 512 : base + LW + (nb + 1) * 512],
                                start=False,
                                stop=(tl == csz - 1),
                            )
                t += csz

            # psA's eviction is hidden behind psB's matmul work; only psB's
            # eviction sits in the tail. Both are emitted after every chunk
            # issue (see note above). The 1-element dependency copies from
            # the LAST chunk's buffer hold psA's stores until the whole
            # input stream has landed: its HBM writes would otherwise
            # interleave with the input stream's tail exactly when the
            # PE-supply margin is thinnest (431 vs 432 ns per K-tile).
            nc.scalar.copy(o_scr[:, 0:1], w_sb[0:1, 0:1])
            nc.vector.tensor_copy(o_scr[:, 1:2], w_sb[0:1, 1:2])
            emit_evict(psA, 0, tail=False)
            emit_evict(psB, N, tail=True)
    # Run Bacc's compile pipeline (wait legalization, register allocation).
    # run_bass_via_pjrt serializes nc.m as-is and never finalizes.
    nc.finalize()
    return nc


def _get_nc():
    global _NC_CACHE
    if _NC_CACHE is None:
        _NC_CACHE = _build_nc()
    return _NC_CACHE


def _prepare_in_maps(inputs: np.ndarray, W: np.ndarray, bias: np.ndarray):
    """Fold softmax(bias) into W, split precision, pack K-tile-major."""
    x = np.asarray(inputs, dtype=np.float32)
    Wf = np.asarray(W, dtype=np.float32)
    b = np.asarray(bias, dtype=np.float32)[0, :, :, 0, 0]          # [I, J]

    # softmax over J per input capsule i (fp32, matches jax.nn.softmax).
    m = b.max(axis=1, keepdims=True)
    e = np.exp(b - m)
    c = e / e.sum(axis=1, keepdims=True)                            # [I, J]

    # Wc[(i,p),(j,d)] = W[i,j,p,d] * c[i,j]  ->  [K, N]
    wc = (Wf.transpose(0, 2, 1, 3) * c[:, None, :, None]).reshape(K, N)
    xT = np.ascontiguousarray(x.reshape(B, K).T)                    # [K, B]

    packed = np.empty((K, TC), dtype=NP_DTYPE)
    if MODE == "e3m4":
        xh = xT.astype(NP_DTYPE)
        resid = (xT - xh.astype(np.float32)) * np.float32(XL_SCALE)
        packed[:, 0:B] = xh
        packed[:, B : 2 * B] = resid.astype(NP_DTYPE)
        ws = np.clip(wc * np.float32(W_SCALE), -15.5, 15.5)
        packed[:, LW:] = ws.astype(NP_DTYPE)
    else:
        packed[:, 0:B] = xT.astype(NP_DTYPE)
        packed[:, LW:] = (wc * np.float32(W_SCALE)).astype(NP_DTYPE)

    in_maps = []
    for cid in range(N_CORES):
        sl = slice(cid * K_CORE, (cid + 1) * K_CORE)
        # K-tile-major packing: [NKT, KT, TC] -> [KT, NKT*TC]
        core = np.ascontiguousarray(
            packed[sl].reshape(NKT, KT, TC).swapaxes(0, 1).reshape(KT, NKT * TC)
        )
        in_maps.append({"wx": core})
    return in_maps


def _squash(s: np.ndarray) -> np.ndarray:
    s2 = np.sum(np.square(s), axis=-1, keepdims=True, dtype=np.float32)
    scale = s2 / (1.0 + s2) / np.sqrt(s2)
    return (scale * s).astype(np.float32)


def run(inputs, W, bias, **spmd_kwargs):
    """Full pipeline; returns (output, BassKernelResults)."""
    in_maps = _prepare_in_maps(inputs, W, bias)
    try:
        res = run_bass_kernel_spmd(
            _get_nc(), in_maps, core_ids=list(range(N_CORES)), **spmd_kwargs
        )
    except Exception:
        # A crashed prior process can leave a core wedged
        # (NRT_EXEC_UNIT_UNRECOVERABLE); one retry clears it.
        import time
        time.sleep(2.0)
        res = run_bass_kernel_spmd(
            _get_nc(), in_maps, core_ids=list(range(N_CORES)), **spmd_kwargs
        )
    s = np.zeros((LW, N), dtype=np.float32)
    for r in res.results:
        o = np.asarray(r["out"], dtype=np.float32)
        s += o[:, 0:N] + o[:, N : 2 * N]
    if MODE == "e3m4":
        s = s[0:B] + s[B : 2 * B] / np.float32(XL_SCALE)
    s /= np.float32(W_SCALE)
    out = _squash(s.reshape(B, J, D))
    return out, res


def kernel(inputs, W, bias):
    out, _ = run(inputs, W, bias)
    return out
